# revision 6
# baseline (speedup 1.0000x reference)
"""Trainium2 Bass kernel for the soft-logic-gate CA problem.

Math (per sample, grid 128x128, 4 layers):
  state' = clip( sum_m sigmoid(tg[l,m]) * prod_j g(bit_j(m), tap_j), 0, 1 )
  taps: A=state[x,y], B=state[x,y+1], C=state[x+1,y], D=state[x+1,y+1] (periodic)
  g(0,t)=1-t, g(1,t)=t;  m = bA*8 + bB*4 + bC*2 + bD.

This is 4-D multilinear interpolation of the 16 gate maps at corner
(A,B,C,D).  We convert the sigmoided gates to multilinear-polynomial
coefficients with an in-place Moebius transform (c[m] -= c[m-bit]) and
evaluate with a Horner butterfly of fused tensor_tensor ops:
  u_i = c[2i] + c[2i+1]*D ; v_j = u_2j + u_{2j+1}*C ; w_k = ... ; s = w0 + w1*A

Sharding: batch 32 -> 8 cores x 4 samples (gates replicated).
Layout per core: partition = grid row (128), free = (sample b:4, col y:128).
Coefficients stay compact (one copy); batch broadcast via stride-0 APs.

Layer 0 is specialized: the initial state has x in even columns and 0 in
odd columns, so even outputs interpolate only over (A,C) and odd outputs
only over (B,D), each a 2-D interpolation on half the grid.
Layer 3 computes only even output columns (the unembed slice).
"""

import numpy as np

import concourse.bacc as bacc
import concourse.mybir as mybir
from concourse.tile import TileContext
from concourse.bass_utils import run_bass_kernel_spmd

F32 = mybir.dt.float32
AL = mybir.AluOpType
P = 128          # partitions = grid rows
B = 4            # samples per core
Y = 128          # grid cols
K = 64           # x cols (even grid cols)
L = 4            # layers
M = 16           # gate combos
N_CORES = 8

# engine for the Moebius transform of layers 1..3 ('g' = gpsimd, 'v' = vector)
MOBIUS_ENG = ("v", "g", "g", "g")


def _emit(tc, nc, x_ap, tg_ap, out_ap_d):
    sync = nc.sync
    vec = nc.vector
    eng_of = {"v": nc.vector, "g": nc.gpsimd}

    def rowshift(dst, src):
        # dst[p, :] = src[(p+1) % 128, :]
        sync.dma_start(out=dst[0:127, :], in_=src[1:128, :])
        sync.dma_start(out=dst[127:128, :], in_=src[0:1, :])

    def colshift(dst, src, w):
        # per sample block of width w: dst[., b, y] = src[., b, (y+1) % w]
        d = dst[:].rearrange("p (b y) -> p b y", b=B)
        s = src[:].rearrange("p (b y) -> p b y", b=B)
        sync.dma_start(out=d[:, :, 0 : w - 1], in_=s[:, :, 1:w])
        sync.dma_start(out=d[:, :, w - 1 : w], in_=s[:, :, 0:1])

    def bcast_c(c):  # coeff (p, n, w) -> (p, n, B, w)
        n, w = c.shape[1], c.shape[2]
        return c.unsqueeze(2).broadcast_to((P, n, B, w))

    def bcast_t(t, n):  # tap (p, B, w) -> (p, n, B, w)
        w = t.shape[2]
        return t.unsqueeze(1).broadcast_to((P, n, B, w))

    with (
        tc.tile_pool(name="coef", bufs=1) as pc,
        tc.tile_pool(name="io", bufs=2) as pio,
        tc.tile_pool(name="st", bufs=2) as pst,
        tc.tile_pool(name="wk", bufs=1) as pwk,
    ):
        tw = pc.tile([P, L * M * Y], F32, tag="tw")  # coeffs, all layers

        # ---- load gates, sigmoid per layer ----
        tga = tg_ap
        for l in range(L):
            tgraw = pio.tile([P, M * Y], F32, tag="tgraw")
            sync.dma_start(
                out=tgraw[:].rearrange("p (m y) -> p m y", m=M),
                in_=tga[l].transpose([1, 0, 2]),
            )
            nc.scalar.activation(
                out=tw[:, l * M * Y : (l + 1) * M * Y],
                in_=tgraw[:],
                func=mybir.ActivationFunctionType.Sigmoid,
            )

        def twl(l):
            return tw[:, l * M * Y : (l + 1) * M * Y]

        # ---- Moebius transform ----
        def mobius_full(l, eng, even_only=False):
            # for each bit step s: c[m] -= c[m - s] for all m with bit set
            for s in (1, 2, 4, 8):
                hi = 8 // s
                v = twl(l).rearrange(
                    "p (hi two lo y) -> p hi two lo y", hi=hi, two=2, lo=s
                )
                dst = v[:, :, 1]
                src = v[:, :, 0]
                if even_only:
                    dst = dst.rearrange("p hi lo (k t) -> p hi lo k t", t=2)[
                        :, :, :, :, 0
                    ]
                    src = src.rearrange("p hi lo (k t) -> p hi lo k t", t=2)[
                        :, :, :, :, 0
                    ]
                eng.tensor_tensor(out=dst, in0=dst, in1=src, op=AL.subtract)

        def mobius_l0():
            # even-y positions use m in {0,2,8,10} (A,C); odd use {0,1,4,5} (B,D)
            t0 = twl(0).rearrange("p (m k t) -> p m k t", m=M, t=2)

            def sub(md, ms, par):
                d = t0[:, md, :, par]
                s = t0[:, ms, :, par]
                vec.tensor_tensor(out=d, in0=d, in1=s, op=AL.subtract)

            sub(2, 0, 0)
            sub(10, 8, 0)
            sub(8, 0, 0)
            sub(10, 2, 0)
            sub(1, 0, 1)
            sub(5, 4, 1)
            sub(4, 0, 1)
            sub(5, 1, 1)

        mobius_l0()
        for l in (1, 2):
            mobius_full(l, eng_of[MOBIUS_ENG[l]])
        mobius_full(3, eng_of[MOBIUS_ENG[3]], even_only=True)

        # ---- layer 0 (specialized 2-D interp) ----
        X = pwk.tile([P, B * K], F32, tag="X")
        Xr = pwk.tile([P, B * K], F32, tag="Xr")
        Xc = pwk.tile([P, B * K], F32, tag="Xc")
        Xrc = pwk.tile([P, B * K], F32, tag="Xrc")
        sync.dma_start(
            out=X[:].rearrange("p (b k) -> p b k", b=B),
            in_=x_ap.transpose([1, 0, 2]),
        )
        rowshift(Xr, X)
        colshift(Xc, X, K)
        colshift(Xrc, Xr, K)

        st1 = pst.tile([P, B * Y], F32, tag="state")
        st1v = st1[:].rearrange("p (b k t) -> p b k t", b=B, t=2)
        ue = pwk.tile([P, 2 * B * K], F32, tag="ue")
        te = pwk.tile([P, B * K], F32, tag="te")
        uev = ue[:].rearrange("p (s b k) -> p s b k", s=2, b=B)
        tev = te[:].rearrange("p (b k) -> p b k", b=B)

        # coeff selectors for layer 0 (at y parity par): pairs (hi-var, lo-var)
        c8 = twl(0).rearrange("p (mh ml k t) -> p mh ml k t", mh=2, ml=8, t=2)
        c4 = twl(0).rearrange("p (mh ml k t) -> p mh ml k t", mh=4, ml=4, t=2)

        def l0_half(par, chi, clo, t_hi, t_out):
            # s = (c_lo0 + c_lo1 * t_hi) + t_out * (c_hi0 + c_hi1 * t_hi)
            # chi = coeff pair AP (p,2,64) for [with-outer-bit], clo likewise
            vec.tensor_tensor(out=uev, in0=chi, in1=bcast_t(t_hi, 2), op=AL.mult)
            vec.tensor_tensor(out=uev, in0=uev, in1=clo, op=AL.add)
            vec.tensor_tensor(out=tev, in0=uev[:, 1], in1=t_out, op=AL.mult)
            vec.tensor_tensor(out=tev, in0=tev, in1=uev[:, 0], op=AL.add)
            vec.tensor_scalar(
                out=st1v[:, :, :, par],
                in0=tev,
                scalar1=0.0,
                scalar2=1.0,
                op0=AL.max,
                op1=AL.min,
            )

        Xv = X[:].rearrange("p (b k) -> p b k", b=B)
        Xrv = Xr[:].rearrange("p (b k) -> p b k", b=B)
        Xcv = Xc[:].rearrange("p (b k) -> p b k", b=B)
        Xrcv = Xrc[:].rearrange("p (b k) -> p b k", b=B)
        # even: taps A=X (bit 8), C=Xr (bit 2): m {0,2,8,10}
        l0_half(
            0,
            bcast_c(c8[:, :, 2, :, 0]),  # {2,10}: * Xr
            bcast_c(c8[:, :, 0, :, 0]),  # {0,8}
            Xrv,
            Xv,
        )
        # odd: taps B=Xc (bit 4), D=Xrc (bit 1): m {0,1,4,5}
        l0_half(
            1,
            bcast_c(c4[:, 0:2, 1, :, 1]),  # {1,5}: * Xrc
            bcast_c(c4[:, 0:2, 0, :, 1]),  # {0,4}
            Xrcv,
            Xcv,
        )

        # ---- generic layer evaluation ----
        u = pwk.tile([P, 8 * B * Y], F32, tag="u")
        v = pwk.tile([P, 4 * B * Y], F32, tag="v")
        w2 = pwk.tile([P, 2 * B * Y], F32, tag="w2")
        tt = pwk.tile([P, B * Y], F32, tag="tt")

        def eval_layer(l, tA, tB_, tC, tD, out_ap, w, engs):
            # taps: APs (p, B, w); out_ap: (p, B, w); w=128 full, 64 even-out
            if w == Y:
                cD = twl(l).rearrange("p (i two y) -> p i two y", i=8, two=2)
                c_odd, c_even = cD[:, :, 1], cD[:, :, 0]
            else:
                cD = twl(l).rearrange(
                    "p (i two k t) -> p i two k t", i=8, two=2, t=2
                )
                c_odd, c_even = cD[:, :, 1, :, 0], cD[:, :, 0, :, 0]
            uv = u[:, : 8 * B * w].rearrange("p (i b y) -> p i b y", i=8, b=B)
            e = engs
            e[0].tensor_tensor(out=uv, in0=bcast_c(c_odd), in1=bcast_t(tD, 8), op=AL.mult)
            e[1].tensor_tensor(out=uv, in0=uv, in1=bcast_c(c_even), op=AL.add)
            uc = u[:, : 8 * B * w].rearrange(
                "p (j two b y) -> p j two b y", j=4, two=2, b=B
            )
            vv = v[:, : 4 * B * w].rearrange("p (j b y) -> p j b y", j=4, b=B)
            e[2].tensor_tensor(out=vv, in0=uc[:, :, 1], in1=bcast_t(tC, 4), op=AL.mult)
            e[3].tensor_tensor(out=vv, in0=vv, in1=uc[:, :, 0], op=AL.add)
            vc = v[:, : 4 * B * w].rearrange(
                "p (j two b y) -> p j two b y", j=2, two=2, b=B
            )
            wv = w2[:, : 2 * B * w].rearrange("p (j b y) -> p j b y", j=2, b=B)
            e[4].tensor_tensor(out=wv, in0=vc[:, :, 1], in1=bcast_t(tB_, 2), op=AL.mult)
            e[5].tensor_tensor(out=wv, in0=wv, in1=vc[:, :, 0], op=AL.add)
            tv = tt[:, : B * w].rearrange("p (b y) -> p b y", b=B)
            e[6].tensor_tensor(out=tv, in0=wv[:, 1], in1=tA, op=AL.mult)
            e[7].tensor_tensor(out=tv, in0=tv, in1=wv[:, 0], op=AL.add)
            e[8].tensor_scalar(
                out=out_ap, in0=tv, scalar1=0.0, scalar2=1.0, op0=AL.max, op1=AL.min
            )

        VE = [vec] * 9

        # ---- layers 1, 2 ----
        st = st1
        for l in (1, 2):
            sB = pst.tile([P, B * Y], F32, tag="sB")
            sC = pst.tile([P, B * Y], F32, tag="sC")
            sD = pst.tile([P, B * Y], F32, tag="sD")
            colshift(sB, st, Y)
            rowshift(sC, st)
            colshift(sD, sC, Y)
            stn = pst.tile([P, B * Y], F32, tag="state")
            bv = lambda t: t[:].rearrange("p (b y) -> p b y", b=B)
            eval_layer(l, bv(st), bv(sB), bv(sC), bv(sD), bv(stn), Y, VE)
            st = stn

        # ---- layer 3 (even outputs only) ----
        sC = pst.tile([P, B * Y], F32, tag="sC")
        rowshift(sC, st)
        stv = st[:].rearrange("p (b k t) -> p b k t", b=B, t=2)
        scv = sC[:].rearrange("p (b k t) -> p b k t", b=B, t=2)
        out_t = pwk.tile([P, B * K], F32, tag="out")
        ov = out_t[:].rearrange("p (b k) -> p b k", b=B)
        eval_layer(
            3,
            stv[:, :, :, 0],
            stv[:, :, :, 1],
            scv[:, :, :, 0],
            scv[:, :, :, 1],
            ov,
            K,
            VE,
        )
        sync.dma_start(out=out_ap_d.transpose([1, 0, 2]), in_=ov)


_NC_CACHE = {}


def build():
    if "nc" in _NC_CACHE:
        return _NC_CACHE["nc"]
    nc = bacc.Bacc(
        "TRN2",
        target_bir_lowering=False,
        debug=False,
        enable_asserts=False,
        num_devices=N_CORES,
    )
    x_d = nc.dram_tensor("x", (B, P, K), F32, kind="ExternalInput")
    tg_d = nc.dram_tensor("tg", (L, M, P, Y), F32, kind="ExternalInput")
    out_d = nc.dram_tensor("out", (B, P, K), F32, kind="ExternalOutput")
    with TileContext(nc) as tc:
        _emit(tc, nc, x_d.ap(), tg_d.ap(), out_d.ap())
    nc.compile()
    _NC_CACHE["nc"] = nc
    return nc


def make_in_maps(x, toggle_gates):
    x = np.ascontiguousarray(np.asarray(x, dtype=np.float32))
    tg = np.ascontiguousarray(np.asarray(toggle_gates, dtype=np.float32))
    return [
        {"x": x[c * B : (c + 1) * B], "tg": tg} for c in range(N_CORES)
    ]


def kernel(x, toggle_gates):
    nc = build()
    res = run_bass_kernel_spmd(
        nc, make_in_maps(x, toggle_gates), core_ids=list(range(N_CORES))
    )
    return np.concatenate([res.results[c]["out"] for c in range(N_CORES)], axis=0)


# revision 9
# speedup vs baseline: 1.9296x; 1.9296x over previous
"""Trainium2 Bass kernel for the soft-logic-gate CA problem.

Math (per sample, grid 128x128, 4 layers):
  state' = clip( sum_m sigmoid(tg[l,m]) * prod_j g(bit_j(m), tap_j), 0, 1 )
  taps: A=state[x,y], B=state[x,y+1], C=state[x+1,y], D=state[x+1,y+1] (periodic)
  g(0,t)=1-t, g(1,t)=t;  m = bA*8 + bB*4 + bC*2 + bD.

This is 4-D multilinear interpolation of the 16 gate maps at corner
(A,B,C,D).  Sigmoided gates are converted to multilinear-polynomial
coefficients with an in-place Moebius transform (c[m] -= c[m-bit]) and
each layer is evaluated with a Horner butterfly of fused tensor_tensor
ops, contracting A first (its tap needs no shift), then B, C, D:
  u_i = c[i] + c[8+i]*A ; v_j = u_j + u_{4+j}*B ; w_k = ... ; s = w0 + w1*D

Sharding: batch 32 -> 8 cores x 4 samples (gates replicated).
Layout per core: partition = grid row (128), free = (sample b:4, col y:128).
Coefficients stay compact (one copy); batch broadcast via stride-0 APs.
Row shifts (x+1) run on the idle TensorE as a permutation matmul into
PSUM, then ScalarE copies back to SBUF; col shifts (y+1) are cheap
same-partition DMAs.  GpSimd is left idle on purpose: its SBUF port is
shared with VectorE and concurrent use slows DVE ~4-6x (measured).

Layer 0 is specialized: the initial state has x in even columns and 0 in
odd columns, so even outputs interpolate only over (A,C) and odd outputs
only over (B,D).  Layer 3 computes only even output columns (the
unembed slice).
"""

import numpy as np

import concourse.bacc as bacc
import concourse.mybir as mybir
from concourse.tile import TileContext
from concourse.bass_utils import run_bass_kernel_spmd

F32 = mybir.dt.float32
DT = mybir.dt.float32  # compute dtype for coeffs/state (float16 variant OK)
AL = mybir.AluOpType
P = 128          # partitions = grid rows
B = 4            # samples per core
Y = 128          # grid cols
K = 64           # x cols (even grid cols)
L = 4            # layers
M = 16           # gate combos
N_CORES = 8


def _emit(tc, nc, x_ap, tg_ap, ps_ap, out_ap_d):
    sync, vec, act, ten = nc.sync, nc.vector, nc.scalar, nc.tensor
    SIG = mybir.ActivationFunctionType.Sigmoid

    def colshift(dst, src, w, engines):
        # per sample block of width w: dst[., b, y] = src[., b, (y+1) % w]
        d = dst[:].rearrange("p (b y) -> p b y", b=B)
        s = src[:].rearrange("p (b y) -> p b y", b=B)
        engines[0].dma_start(out=d[:, :, 0 : w - 1], in_=s[:, :, 1:w])
        engines[1].dma_start(out=d[:, :, w - 1 : w], in_=s[:, :, 0:1])

    def bcast_c(c):  # coeff (p, n, w) -> (p, n, B, w)
        n, w = c.shape[1], c.shape[2]
        return c.unsqueeze(2).broadcast_to((P, n, B, w))

    def bcast_t(t, n):  # tap (p, B, w) -> (p, n, B, w)
        w = t.shape[2]
        return t.unsqueeze(1).broadcast_to((P, n, B, w))

    with (
        tc.tile_pool(name="coef", bufs=1) as pc,
        tc.tile_pool(name="io", bufs=2) as pio,
        tc.tile_pool(name="st", bufs=2) as pst,
        tc.tile_pool(name="wk", bufs=1) as pwk,
        tc.tile_pool(name="ps", bufs=2, space="PSUM") as pps,
    ):
        # ---- loads ----
        psh = pwk.tile([P, P], DT, tag="psh")
        sync.dma_start(out=psh[:], in_=ps_ap)
        X32 = pwk.tile([P, B * K], F32, tag="X32")
        sync.dma_start(
            out=X32[:].rearrange("p (b k) -> p b k", b=B),
            in_=x_ap.transpose([1, 0, 2]),
        )
        tw = pc.tile([P, L * M * Y], DT, tag="tw")  # coeffs, all layers
        tga = tg_ap

        def twl(l):
            return tw[:, l * M * Y : (l + 1) * M * Y]

        tgraw0 = pio.tile([P, M * Y], F32, tag="tgraw")
        sync.dma_start(
            out=tgraw0[:].rearrange("p (m y) -> p m y", m=M),
            in_=tga[0].transpose([1, 0, 2]),
        )

        if DT != F32:
            X = pwk.tile([P, B * K], DT, tag="X")
            act.copy(out=X[:], in_=X32[:])
        else:
            X = X32

        # layer-0 taps: Xr = rowshift(X) via PE; Xc = colshift(X); Xrc = rowshift(Xc)
        Xc = pwk.tile([P, B * K], DT, tag="Xc")
        colshift(Xc, X, K, [sync, sync])
        pXr = pps.tile([P, B * K], F32, tag="pXr")
        pXrc = pps.tile([P, B * K], F32, tag="pXrc")
        ten.matmul(pXr[:], psh[:], X[:], start=True, stop=True)
        ten.matmul(pXrc[:], psh[:], Xc[:], start=True, stop=True)
        Xr = pwk.tile([P, B * K], DT, tag="Xr")
        Xrc = pwk.tile([P, B * K], DT, tag="Xrc")
        act.copy(out=Xr[:], in_=pXr[:])
        act.copy(out=Xrc[:], in_=pXrc[:])

        # layer-0 sigmoid: only the 8 coefficient maps layer 0 reads.
        # m = a*8 + bb*4 + c*2 + d; even-y outputs use m {0,2,8,10} (bb=d=0);
        # odd-y use m {0,1,4,5} (a=c=0).
        bits = "p (a bb c d k t) -> p a bb c d k t"
        t0raw = tgraw0[:].rearrange(bits, a=2, bb=2, c=2, d=2, t=2)
        t0c = twl(0).rearrange(bits, a=2, bb=2, c=2, d=2, t=2)
        act.activation(
            out=t0c[:, :, 0, :, 0, :, 0], in_=t0raw[:, :, 0, :, 0, :, 0], func=SIG
        )
        act.activation(
            out=t0c[:, 0, :, 0, :, :, 1], in_=t0raw[:, 0, :, 0, :, :, 1], func=SIG
        )

        # remaining layers: full sigmoid
        for l in range(1, L):
            tgraw = pio.tile([P, M * Y], F32, tag="tgraw")
            sync.dma_start(
                out=tgraw[:].rearrange("p (m y) -> p m y", m=M),
                in_=tga[l].transpose([1, 0, 2]),
            )
            act.activation(out=twl(l), in_=tgraw[:], func=SIG)

        # ---- Moebius transforms (all on VectorE) ----
        def mobius_l0():
            t0 = twl(0).rearrange("p (m k t) -> p m k t", m=M, t=2)

            def sub(md, ms, par):
                d = t0[:, md, :, par]
                s = t0[:, ms, :, par]
                vec.tensor_tensor(out=d, in0=d, in1=s, op=AL.subtract)

            sub(2, 0, 0)
            sub(10, 8, 0)
            sub(8, 0, 0)
            sub(10, 2, 0)
            sub(1, 0, 1)
            sub(5, 4, 1)
            sub(4, 0, 1)
            sub(5, 1, 1)

        def mobius_full(l, even_only=False):
            for s in (1, 2, 4, 8):
                hi = 8 // s
                v = twl(l).rearrange(
                    "p (hi two lo y) -> p hi two lo y", hi=hi, two=2, lo=s
                )
                dst = v[:, :, 1]
                src = v[:, :, 0]
                if even_only:
                    dst = dst.rearrange("p hi lo (k t) -> p hi lo k t", t=2)[
                        :, :, :, :, 0
                    ]
                    src = src.rearrange("p hi lo (k t) -> p hi lo k t", t=2)[
                        :, :, :, :, 0
                    ]
                vec.tensor_tensor(out=dst, in0=dst, in1=src, op=AL.subtract)

        # ---- layer 0 eval (A-first 2-D interp) ----
        mobius_l0()

        st1 = pst.tile([P, B * Y], DT, tag="state")
        st1v = st1[:].rearrange("p (b k t) -> p b k t", b=B, t=2)
        ue = pwk.tile([P, 2 * B * K], DT, tag="ue")
        te = pwk.tile([P, B * K], DT, tag="te")
        uev = ue[:].rearrange("p (s b k) -> p s b k", s=2, b=B)
        tev = te[:].rearrange("p (b k) -> p b k", b=B)

        def l0_half(par, chi, clo, t_in, t_out):
            # s = (c_lo0 + c_hi0 * t_in) + t_out * (c_lo1 + c_hi1 * t_in)
            vec.tensor_tensor(out=uev, in0=chi, in1=bcast_t(t_in, 2), op=AL.mult)
            vec.tensor_tensor(out=uev, in0=uev, in1=clo, op=AL.add)
            vec.tensor_tensor(out=tev, in0=uev[:, 1], in1=t_out, op=AL.mult)
            vec.tensor_tensor(out=tev, in0=tev, in1=uev[:, 0], op=AL.add)
            vec.tensor_scalar(
                out=st1v[:, :, :, par],
                in0=tev,
                scalar1=0.0,
                scalar2=1.0,
                op0=AL.max,
                op1=AL.min,
            )

        bv = lambda t: t[:].rearrange("p (b k) -> p b k", b=B)
        v0 = twl(0).rearrange(bits, a=2, bb=2, c=2, d=2, t=2)
        # even: s = (c0 + c8*X) + Xr*(c2 + c10*X);  pairs {0,2} and {8,10}
        l0_half(
            0,
            bcast_c(v0[:, 1, 0, :, 0, :, 0]),  # {8,10}: * X
            bcast_c(v0[:, 0, 0, :, 0, :, 0]),  # {0,2}
            bv(X),
            bv(Xr),
        )
        # odd: s = (c0 + c4*Xc) + Xrc*(c1 + c5*Xc);  pairs {0,1} and {4,5}
        l0_half(
            1,
            bcast_c(v0[:, 0, 1, 0, :, :, 1]),  # {4,5}: * Xc
            bcast_c(v0[:, 0, 0, 0, :, :, 1]),  # {0,1}
            bv(Xc),
            bv(Xrc),
        )

        # ---- generic layer evaluation (A-first) ----
        u = pwk.tile([P, 8 * B * Y], DT, tag="u")
        v_t = pwk.tile([P, 4 * B * Y], DT, tag="v")
        w2 = pwk.tile([P, 2 * B * Y], DT, tag="w2")
        tt = pwk.tile([P, B * Y], DT, tag="tt")

        def eval_layer(l, tA, tB_, tC, tD, out_ap, w):
            # taps: APs (p, B, w); out_ap: (p, B, w); w=128 full, 64 even-out
            if w == Y:
                cv = twl(l).rearrange("p (two i y) -> p two i y", two=2, i=8)
                cHI, cLO = cv[:, 1], cv[:, 0]  # (p, 8, 128)
            else:
                cv = twl(l).rearrange(
                    "p (two i k t) -> p two i k t", two=2, i=8, t=2
                )
                cHI, cLO = cv[:, 1, :, :, 0], cv[:, 0, :, :, 0]  # (p, 8, 64)
            uv = u[:, : 8 * B * w].rearrange("p (i b y) -> p i b y", i=8, b=B)
            vec.tensor_tensor(out=uv, in0=bcast_c(cHI), in1=bcast_t(tA, 8), op=AL.mult)
            vec.tensor_tensor(out=uv, in0=uv, in1=bcast_c(cLO), op=AL.add)
            uc = u[:, : 8 * B * w].rearrange(
                "p (two j b y) -> p two j b y", two=2, j=4, b=B
            )
            vv = v_t[:, : 4 * B * w].rearrange("p (j b y) -> p j b y", j=4, b=B)
            vec.tensor_tensor(out=vv, in0=uc[:, 1], in1=bcast_t(tB_, 4), op=AL.mult)
            vec.tensor_tensor(out=vv, in0=vv, in1=uc[:, 0], op=AL.add)
            vc = v_t[:, : 4 * B * w].rearrange(
                "p (two j b y) -> p two j b y", two=2, j=2, b=B
            )
            wv = w2[:, : 2 * B * w].rearrange("p (j b y) -> p j b y", j=2, b=B)
            vec.tensor_tensor(out=wv, in0=vc[:, 1], in1=bcast_t(tC, 2), op=AL.mult)
            vec.tensor_tensor(out=wv, in0=wv, in1=vc[:, 0], op=AL.add)
            wc = w2[:, : 2 * B * w].rearrange("p (two b y) -> p two b y", two=2, b=B)
            tv = tt[:, : B * w].rearrange("p (b y) -> p b y", b=B)
            vec.tensor_tensor(out=tv, in0=wc[:, 1], in1=tD, op=AL.mult)
            vec.tensor_tensor(out=tv, in0=tv, in1=wc[:, 0], op=AL.add)
            vec.tensor_scalar(
                out=out_ap, in0=tv, scalar1=0.0, scalar2=1.0, op0=AL.max, op1=AL.min
            )

        def rowshifted(src, n, tag):
            # PE permutation matmul + ScalarE copy-back; returns SBUF tile
            pt = pps.tile([P, n], F32, tag="p" + tag)
            ten.matmul(pt[:], psh[:], src[:], start=True, stop=True)
            out = pst.tile([P, n], DT, tag=tag)
            act.copy(out=out[:], in_=pt[:])
            return out

        # ---- layers 1, 2 ----
        st = st1
        for l in (1, 2):
            mobius_full(l)
            sB = pst.tile([P, B * Y], DT, tag="sB")
            colshift(sB, st, Y, [sync, nc.scalar])
            sC = rowshifted(st, B * Y, "sC")
            sD = rowshifted(sB, B * Y, "sD")
            stn = pst.tile([P, B * Y], DT, tag="state")
            bvy = lambda t: t[:].rearrange("p (b y) -> p b y", b=B)
            eval_layer(l, bvy(st), bvy(sB), bvy(sC), bvy(sD), bvy(stn), Y)
            st = stn

        # ---- layer 3 (even outputs only; B/D taps are odd cols of st/sC) ----
        mobius_full(3, even_only=True)
        sC = rowshifted(st, B * Y, "sC")
        stv = st[:].rearrange("p (b k t) -> p b k t", b=B, t=2)
        scv = sC[:].rearrange("p (b k t) -> p b k t", b=B, t=2)
        out_t = pwk.tile([P, B * K], F32, tag="out")
        ov = out_t[:].rearrange("p (b k) -> p b k", b=B)
        eval_layer(
            3,
            stv[:, :, :, 0],
            stv[:, :, :, 1],
            scv[:, :, :, 0],
            scv[:, :, :, 1],
            ov,
            K,
        )
        sync.dma_start(out=out_ap_d.transpose([1, 0, 2]), in_=ov)


_NC_CACHE = {}


def _np_dt():
    return {F32: np.float32, mybir.dt.float16: np.float16}[DT]


def build():
    if "nc" in _NC_CACHE:
        return _NC_CACHE["nc"]
    nc = bacc.Bacc(
        "TRN2",
        target_bir_lowering=False,
        debug=False,
        enable_asserts=False,
        num_devices=N_CORES,
    )
    x_d = nc.dram_tensor("x", (B, P, K), F32, kind="ExternalInput")
    tg_d = nc.dram_tensor("tg", (L, M, P, Y), F32, kind="ExternalInput")
    ps_d = nc.dram_tensor("pshift", (P, P), DT, kind="ExternalInput")
    out_d = nc.dram_tensor("out", (B, P, K), F32, kind="ExternalOutput")
    with TileContext(nc) as tc:
        _emit(tc, nc, x_d.ap(), tg_d.ap(), ps_d.ap(), out_d.ap())
    nc.compile()
    _NC_CACHE["nc"] = nc
    return nc


def _pshift():
    p = np.eye(P, k=-1, dtype=np.float64)
    p[0, P - 1] = 1.0
    return p.astype(_np_dt())


def make_in_maps(x, toggle_gates):
    x = np.ascontiguousarray(np.asarray(x, dtype=np.float32))
    tg = np.ascontiguousarray(np.asarray(toggle_gates, dtype=np.float32))
    psm = _pshift()
    return [
        {"x": x[c * B : (c + 1) * B], "tg": tg, "pshift": psm}
        for c in range(N_CORES)
    ]


def kernel(x, toggle_gates):
    nc = build()
    res = run_bass_kernel_spmd(
        nc, make_in_maps(x, toggle_gates), core_ids=list(range(N_CORES))
    )
    return np.concatenate([res.results[c]["out"] for c in range(N_CORES)], axis=0)


# revision 10
# speedup vs baseline: 2.8913x; 1.4984x over previous
"""Trainium2 Bass kernel for the soft-logic-gate CA problem.

Math (per sample, grid 128x128, 4 layers):
  state' = clip( sum_m sigmoid(tg[l,m]) * prod_j g(bit_j(m), tap_j), 0, 1 )
  taps: A=state[x,y], B=state[x,y+1], C=state[x+1,y], D=state[x+1,y+1] (periodic)
  g(0,t)=1-t, g(1,t)=t;  m = bA*8 + bB*4 + bC*2 + bD.

This is 4-D multilinear interpolation of the 16 gate maps at corner
(A,B,C,D).  Sigmoided gates are converted to multilinear-polynomial
coefficients with an in-place Moebius transform (c[m] -= c[m-bit]) and
each layer is evaluated with a Horner butterfly of fused tensor_tensor
ops, contracting A first (its tap needs no shift), then B, C, D:
  u_i = c[i] + c[8+i]*A ; v_j = u_j + u_{4+j}*B ; w_k = ... ; s = w0 + w1*D

Sharding: batch 32 -> 8 cores x 4 samples (gates replicated).
Layout per core: partition = grid row (128), free = (sample b:4, col y:128).
Coefficients stay compact (one copy); batch broadcast via stride-0 APs.
Row shifts (x+1) run on the idle TensorE as a permutation matmul into
PSUM, then ScalarE copies back to SBUF; col shifts (y+1) are cheap
same-partition DMAs.  GpSimd is left idle on purpose: its SBUF port is
shared with VectorE and concurrent use slows DVE ~4-6x (measured).

Compute dtype is fp16 (DVE 2x_1P mode on unit-stride ops; ~1.7e-3 rel
err vs fp32 reference, measured).  Layer 0 reads only 8 gate maps (the
initial state has x in even columns, 0 in odd), and layer 3 computes
only even output columns; both use de-interleaved compact coefficient
blocks so every hot op keeps unit stride.
"""

import numpy as np

import concourse.bacc as bacc
import concourse.mybir as mybir
from concourse.tile import TileContext
from concourse.bass_utils import run_bass_kernel_spmd

F32 = mybir.dt.float32
DT = mybir.dt.float16  # compute dtype (float32 also works)
AL = mybir.AluOpType
P = 128          # partitions = grid rows
B = 4            # samples per core
Y = 128          # grid cols
K = 64           # x cols (even grid cols)
L = 4            # layers
M = 16           # gate combos
N_CORES = 8


def _emit(tc, nc, x_ap, tg_ap, ps_ap, out_ap_d):
    sync, vec, act, ten = nc.sync, nc.vector, nc.scalar, nc.tensor
    SIG = mybir.ActivationFunctionType.Sigmoid

    def colshift(dst, src, w, engines):
        # per sample block of width w: dst[., b, y] = src[., b, (y+1) % w]
        d = dst[:].rearrange("p (b y) -> p b y", b=B)
        s = src[:].rearrange("p (b y) -> p b y", b=B)
        engines[0].dma_start(out=d[:, :, 0 : w - 1], in_=s[:, :, 1:w])
        engines[1].dma_start(out=d[:, :, w - 1 : w], in_=s[:, :, 0:1])

    def bcast_c(c):  # coeff (p, n, w) -> (p, n, B, w)
        n, w = c.shape[1], c.shape[2]
        return c.unsqueeze(2).broadcast_to((P, n, B, w))

    def bcast_t(t, n):  # tap (p, B, w) -> (p, n, B, w)
        w = t.shape[2]
        return t.unsqueeze(1).broadcast_to((P, n, B, w))

    def clamp(out_ap, in_ap):
        vec.tensor_scalar(
            out=out_ap, in0=in_ap, scalar1=0.0, scalar2=1.0, op0=AL.max, op1=AL.min
        )

    with (
        tc.tile_pool(name="coef", bufs=1) as pc,
        tc.tile_pool(name="io", bufs=2) as pio,
        tc.tile_pool(name="st", bufs=2) as pst,
        tc.tile_pool(name="wk", bufs=1) as pwk,
        tc.tile_pool(name="ps", bufs=2, space="PSUM") as pps,
    ):
        # preload the sigmoid ACT table while DMAs run
        scr = pwk.tile([P, 1], F32, tag="scr")
        vec.memset(scr[:], 0.0)
        act.activation(out=scr[:], in_=scr[:], func=SIG)

        # ---- loads (tg0 first: it gates the layer-0 coefficient chain) ----
        tw = pc.tile([P, L * M * Y], DT, tag="tw")  # coeffs, all layers
        tga = tg_ap

        def twl(l):
            return tw[:, l * M * Y : (l + 1) * M * Y]

        tgraw0 = pio.tile([P, M * Y], F32, tag="tgraw")
        sync.dma_start(
            out=tgraw0[:].rearrange("p (m y) -> p m y", m=M),
            in_=tga[0].transpose([1, 0, 2]),
        )
        psh = pwk.tile([P, P], DT, tag="psh")
        sync.dma_start(out=psh[:], in_=ps_ap)
        X32 = pwk.tile([P, B * K], F32, tag="X32")
        sync.dma_start(
            out=X32[:].rearrange("p (b k) -> p b k", b=B),
            in_=x_ap.transpose([1, 0, 2]),
        )
        if DT != F32:
            X = pwk.tile([P, B * K], DT, tag="X")
            act.copy(out=X[:], in_=X32[:])
        else:
            X = X32

        # layer-0 taps: Xr = rowshift(X) via PE; Xc = colshift(X); Xrc = rowshift(Xc)
        Xc = pwk.tile([P, B * K], DT, tag="Xc")
        colshift(Xc, X, K, [sync, sync])
        pXr = pps.tile([P, B * K], F32, tag="pXr")
        pXrc = pps.tile([P, B * K], F32, tag="pXrc")
        ten.matmul(pXr[:], psh[:], X[:], start=True, stop=True)
        ten.matmul(pXrc[:], psh[:], Xc[:], start=True, stop=True)
        Xr = pwk.tile([P, B * K], DT, tag="Xr")
        Xrc = pwk.tile([P, B * K], DT, tag="Xrc")
        act.copy(out=Xr[:], in_=pXr[:])
        act.copy(out=Xrc[:], in_=pXrc[:])

        # ---- layer-0 coefficients: two compact 4-map blocks ----
        # m = a*8 + bb*4 + c*2 + d.  Even-y outputs use {0,2,8,10} (bb=d=0),
        # odd-y use {0,1,4,5} (a=c=0).  Stored de-interleaved:
        #   ce = twl0[0:256]   = [c0,c2,c8,c10] x k    (order (a,c))
        #   co = twl0[256:512] = [c0,c1,c4,c5]  x k    (order (bb,d))
        bits = "p (a bb c d k t) -> p a bb c d k t"
        t0raw = tgraw0[:].rearrange(bits, a=2, bb=2, c=2, d=2, t=2)
        ce = twl(0)[:, 0:256]
        co = twl(0)[:, 256:512]
        ce4 = ce.rearrange("p (a c k) -> p a c k", a=2, c=2)
        co4 = co.rearrange("p (bb d k) -> p bb d k", bb=2, d=2)
        act.activation(out=ce4, in_=t0raw[:, :, 0, :, 0, :, 0], func=SIG)
        act.activation(out=co4, in_=t0raw[:, 0, :, 0, :, :, 1], func=SIG)
        # 2-D Moebius on each block (2 fused subtract passes each)
        for blk, n4 in ((ce, ce4), (co, co4)):
            d_ = n4[:, :, 1]
            s_ = n4[:, :, 0]
            vec.tensor_tensor(out=d_, in0=d_, in1=s_, op=AL.subtract)
            hi = blk.rearrange("p (h q) -> p h q", h=2)
            vec.tensor_tensor(
                out=hi[:, 1], in0=hi[:, 1], in1=hi[:, 0], op=AL.subtract
            )

        # remaining layers: full sigmoid (layer 3 de-interleaved to even-y)
        for l in range(1, L):
            tgraw = pio.tile([P, M * Y], F32, tag="tgraw")
            sync.dma_start(
                out=tgraw[:].rearrange("p (m y) -> p m y", m=M),
                in_=tga[l].transpose([1, 0, 2]),
            )
            if l < 3:
                act.activation(out=twl(l), in_=tgraw[:], func=SIG)
            else:
                tr = tgraw[:].rearrange("p (m k t) -> p m k t", m=M, t=2)
                c3 = twl(3)[:, 0 : M * K].rearrange("p (m k) -> p m k", m=M)
                act.activation(out=c3, in_=tr[:, :, :, 0], func=SIG)

        def mobius_full(block, w):
            # block: (p, 16*w) coeff AP; in-place c[m] -= c[m-bit] per bit
            for s in (1, 2, 4, 8):
                hi = 8 // s
                v = block.rearrange(
                    "p (hi two lo y) -> p hi two lo y", hi=hi, two=2, lo=s, y=w
                )
                vec.tensor_tensor(
                    out=v[:, :, 1], in0=v[:, :, 1], in1=v[:, :, 0], op=AL.subtract
                )

        # ---- layer 0 eval (A-first 2-D interp) ----
        st1 = pst.tile([P, B * Y], DT, tag="state")
        st1v = st1[:].rearrange("p (b k t) -> p b k t", b=B, t=2)
        ue = pwk.tile([P, 2 * B * K], DT, tag="ue")
        te = pwk.tile([P, B * K], DT, tag="te")
        uev = ue[:].rearrange("p (s b k) -> p s b k", s=2, b=B)
        tev = te[:].rearrange("p (b k) -> p b k", b=B)
        bv = lambda t: t[:].rearrange("p (b k) -> p b k", b=B)

        def l0_half(par, cpair, t_in, t_out):
            # s = (c_lo0 + c_hi0 * t_in) + t_out * (c_lo1 + c_hi1 * t_in)
            cp = cpair.rearrange("p (h s k) -> p h s k", h=2, s=2)
            vec.tensor_tensor(out=uev, in0=bcast_c(cp[:, 1]), in1=bcast_t(t_in, 2), op=AL.mult)
            vec.tensor_tensor(out=uev, in0=uev, in1=bcast_c(cp[:, 0]), op=AL.add)
            vec.tensor_tensor(out=tev, in0=uev[:, 1], in1=t_out, op=AL.mult)
            vec.tensor_tensor(out=tev, in0=tev, in1=uev[:, 0], op=AL.add)
            clamp(st1v[:, :, :, par], tev)

        # even: s = (c0 + c8*X) + Xr*(c2 + c10*X)   (ce = [c0,c2,c8,c10])
        l0_half(0, ce, bv(X), bv(Xr))
        # odd:  s = (c0 + c4*Xc) + Xrc*(c1 + c5*Xc) (co = [c0,c1,c4,c5])
        l0_half(1, co, bv(Xc), bv(Xrc))

        # ---- generic layer evaluation (A-first), returns pre-clamp AP ----
        u = pwk.tile([P, 8 * B * Y], DT, tag="u")
        v_t = pwk.tile([P, 4 * B * Y], DT, tag="v")
        w2 = pwk.tile([P, 2 * B * Y], DT, tag="w2")
        tt = pwk.tile([P, B * Y], DT, tag="tt")

        def eval_layer(cv, tA, tB_, tC, tD, w):
            # cv: (p, two, i, w) coeff view; taps: (p, B, w) APs
            cHI, cLO = cv[:, 1], cv[:, 0]
            uv = u[:, : 8 * B * w].rearrange("p (i b y) -> p i b y", i=8, b=B)
            vec.tensor_tensor(out=uv, in0=bcast_c(cHI), in1=bcast_t(tA, 8), op=AL.mult)
            vec.tensor_tensor(out=uv, in0=uv, in1=bcast_c(cLO), op=AL.add)
            uc = u[:, : 8 * B * w].rearrange(
                "p (two j b y) -> p two j b y", two=2, j=4, b=B
            )
            vv = v_t[:, : 4 * B * w].rearrange("p (j b y) -> p j b y", j=4, b=B)
            vec.tensor_tensor(out=vv, in0=uc[:, 1], in1=bcast_t(tB_, 4), op=AL.mult)
            vec.tensor_tensor(out=vv, in0=vv, in1=uc[:, 0], op=AL.add)
            vc = v_t[:, : 4 * B * w].rearrange(
                "p (two j b y) -> p two j b y", two=2, j=2, b=B
            )
            wv = w2[:, : 2 * B * w].rearrange("p (j b y) -> p j b y", j=2, b=B)
            vec.tensor_tensor(out=wv, in0=vc[:, 1], in1=bcast_t(tC, 2), op=AL.mult)
            vec.tensor_tensor(out=wv, in0=wv, in1=vc[:, 0], op=AL.add)
            wc = w2[:, : 2 * B * w].rearrange("p (two b y) -> p two b y", two=2, b=B)
            tv = tt[:, : B * w].rearrange("p (b y) -> p b y", b=B)
            vec.tensor_tensor(out=tv, in0=wc[:, 1], in1=tD, op=AL.mult)
            vec.tensor_tensor(out=tv, in0=tv, in1=wc[:, 0], op=AL.add)
            return tv

        def rowshifted(src, n, tag):
            # PE permutation matmul + ScalarE copy-back; returns SBUF tile
            pt = pps.tile([P, n], F32, tag="p" + tag)
            ten.matmul(pt[:], psh[:], src[:], start=True, stop=True)
            out = pst.tile([P, n], DT, tag=tag)
            act.copy(out=out[:], in_=pt[:])
            return out

        # ---- layers 1, 2 ----
        st = st1
        bvy = lambda t: t[:].rearrange("p (b y) -> p b y", b=B)
        for l in (1, 2):
            mobius_full(twl(l), Y)
            sB = pst.tile([P, B * Y], DT, tag="sB")
            colshift(sB, st, Y, [sync, nc.scalar])
            sC = rowshifted(st, B * Y, "sC")
            sD = rowshifted(sB, B * Y, "sD")
            cv = twl(l).rearrange("p (two i y) -> p two i y", two=2, i=8)
            tv = eval_layer(cv, bvy(st), bvy(sB), bvy(sC), bvy(sD), Y)
            if l == 1:
                stn = pst.tile([P, B * Y], DT, tag="state")
                clamp(bvy(stn), tv)
            else:
                # layer-3 state stored as parity planes: [even b*k | odd b*k]
                stn = pst.tile([P, B * Y], DT, tag="state")
                tvp = tv.rearrange("p b (k t) -> p b k t", t=2)
                clamp(bv(stn[:, 0 : B * K]), tvp[:, :, :, 0])
                clamp(bv(stn[:, B * K : 2 * B * K]), tvp[:, :, :, 1])
            st = stn

        # ---- layer 3 (even outputs only; compact coeffs, plane taps) ----
        mobius_full(twl(3)[:, 0 : M * K], K)
        sC = rowshifted(st, B * Y, "sC")
        out_t = pwk.tile([P, B * K], F32, tag="out")
        cv3 = twl(3)[:, 0 : M * K].rearrange("p (two i k) -> p two i k", two=2, i=8)
        tv = eval_layer(
            cv3,
            bv(st[:, 0 : B * K]),
            bv(st[:, B * K : 2 * B * K]),
            bv(sC[:, 0 : B * K]),
            bv(sC[:, B * K : 2 * B * K]),
            K,
        )
        ov = out_t[:].rearrange("p (b k) -> p b k", b=B)
        clamp(ov, tv)
        sync.dma_start(out=out_ap_d.transpose([1, 0, 2]), in_=ov)


_NC_CACHE = {}


def _np_dt():
    return {F32: np.float32, mybir.dt.float16: np.float16}[DT]


def build():
    if "nc" in _NC_CACHE:
        return _NC_CACHE["nc"]
    nc = bacc.Bacc(
        "TRN2",
        target_bir_lowering=False,
        debug=False,
        enable_asserts=False,
        num_devices=N_CORES,
    )
    x_d = nc.dram_tensor("x", (B, P, K), F32, kind="ExternalInput")
    tg_d = nc.dram_tensor("tg", (L, M, P, Y), F32, kind="ExternalInput")
    ps_d = nc.dram_tensor("pshift", (P, P), DT, kind="ExternalInput")
    out_d = nc.dram_tensor("out", (B, P, K), F32, kind="ExternalOutput")
    with TileContext(nc) as tc:
        _emit(tc, nc, x_d.ap(), tg_d.ap(), ps_d.ap(), out_d.ap())
    nc.compile()
    _NC_CACHE["nc"] = nc
    return nc


def _pshift():
    p = np.eye(P, k=-1, dtype=np.float64)
    p[0, P - 1] = 1.0
    return p.astype(_np_dt())


def make_in_maps(x, toggle_gates):
    x = np.ascontiguousarray(np.asarray(x, dtype=np.float32))
    tg = np.ascontiguousarray(np.asarray(toggle_gates, dtype=np.float32))
    psm = _pshift()
    return [
        {"x": x[c * B : (c + 1) * B], "tg": tg, "pshift": psm}
        for c in range(N_CORES)
    ]


def kernel(x, toggle_gates):
    nc = build()
    res = run_bass_kernel_spmd(
        nc, make_in_maps(x, toggle_gates), core_ids=list(range(N_CORES))
    )
    return np.concatenate([res.results[c]["out"] for c in range(N_CORES)], axis=0)
